# revision 13
# baseline (speedup 1.0000x reference)
"""Trainium2 Bass kernel for the CSAG (channel squeeze-attention gate) module.

Strategy: pure data parallelism over batch across 8 NeuronCores (4 batches
per core, no collectives). Per core the dominant cost is streaming the
67MB x-shard from HBM; global avg-pool (ScalarE activation accumulate) and
max-pool (VectorE tensor_reduce) run under the DMA shadow. The tiny fc /
groupnorm / attention / sigmoid epilogue runs on-chip per core.

Host-side preprocessing (all tiny): shard x on batch, pre-transpose
fc1_w/fc2_w (contraction dim must sit on partitions), fold the 1/HW mean
scale into fc1_wT (2^-14, exact in fp32), combine biases, and fold the
groupnorm affine into the per-channel q/k/v scales.

Perf notes:
- const/param DMAs go on the scalar HWDGE ring so the x stream starts
  immediately on the sync ring
- the last (b, ct) tile is streamed in tapering chunks so the final
  un-overlapped reduce is small
- every ScalarE function used (exp/copy/square/tanh) lives in the
  exp_and_others table set; a warm-up exp pins that set at t=0 so no
  table reload lands on the critical path (rsqrt is a DVE bit-trick +
  Newton, sigmoid goes through tanh)
"""

import numpy as np
from contextlib import ExitStack

import concourse.bass as bass
import concourse.tile as tile
from concourse import bacc, masks, mybir
from concourse.bass_utils import run_bass_kernel_spmd

f32 = mybir.dt.float32
f32r = mybir.dt.float32r
i32 = mybir.dt.int32
AF = mybir.ActivationFunctionType
ALU = mybir.AluOpType
AX = mybir.AxisListType

B, DIM, H, W = 32, 256, 128, 128
S = H * W                    # 16384 spatial elements
NCORES = 8
BPC = B // NCORES            # 4 batches per core
NCT = DIM // 128             # 2 channel tiles of 128 partitions
CHUNK = 8192
EPS = 1e-5
SCALER = float(DIM) ** -0.5  # head_dim == DIM (1 head)

IN_BUFS = 2


def _tile_chunks(b, ct):
    """Spatial chunk sizes for one (batch, channel-tile); taper the very
    last tile so the final non-overlapped reduction is short."""
    if b == BPC - 1 and ct == NCT - 1:
        return [8192, 4096, 2048, 1024, 1024]
    return [CHUNK, CHUNK]


def _body(ctx, tc, x, fc1t, fc2t, bsum, wqkv, out):
    nc = tc.nc
    const = ctx.enter_context(tc.tile_pool(name="const", bufs=1))
    inp = ctx.enter_context(tc.tile_pool(name="inp", bufs=IN_BUFS))
    scr = ctx.enter_context(tc.tile_pool(name="scr", bufs=1))
    small = ctx.enter_context(tc.tile_pool(name="small", bufs=1))
    tmp = ctx.enter_context(tc.tile_pool(name="tmp", bufs=2))
    etp = ctx.enter_context(tc.tile_pool(name="etp", bufs=2))
    psY = ctx.enter_context(tc.tile_pool(name="psY", bufs=1, space=bass.MemorySpace.PSUM))
    psV = ctx.enter_context(tc.tile_pool(name="psV", bufs=2, space=bass.MemorySpace.PSUM))
    psP = ctx.enter_context(tc.tile_pool(name="psP", bufs=2, space=bass.MemorySpace.PSUM))
    psN = ctx.enter_context(tc.tile_pool(name="psN", bufs=2, space=bass.MemorySpace.PSUM))

    # Pin the exp_and_others activation table set (covers exp/copy/square/
    # tanh) before any real ScalarE work so no set switch hits the epilogue.
    warm = const.tile([1, 1], f32, tag="warm")
    nc.vector.memset(warm[:], 0.0)
    nc.scalar.activation(warm[:], warm[:], AF.Exp)

    # ---- constants / parameters -> SBUF (scalar HWDGE ring; sync ring is
    # reserved for the x stream)
    fc1t_sb, fc2t_sb = [], []
    for ct in range(NCT):
        t1 = const.tile([128, DIM], f32, tag=f"fc1t{ct}")
        t2 = const.tile([128, DIM], f32, tag=f"fc2t{ct}")
        nc.scalar.dma_start(out=t1[:], in_=fc1t[ct * 128:(ct + 1) * 128, :])
        nc.scalar.dma_start(out=t2[:], in_=fc2t[ct * 128:(ct + 1) * 128, :])
        fc1t_sb.append(t1)
        fc2t_sb.append(t2)
    bsum_sb = const.tile([1, DIM], f32, tag="bsum")
    nc.scalar.dma_start(out=bsum_sb[:], in_=bsum[:])
    wqkv_sb = const.tile([BPC, 6 * DIM], f32, tag="wqkv")
    nc.scalar.dma_start(out=wqkv_sb[:], in_=wqkv[:])
    ones14 = const.tile([1, BPC], f32, tag="ones14")
    nc.vector.memset(ones14[:], 1.0)
    ident = const.tile([128, 128], f32, tag="ident")
    masks.make_identity(nc, ident[:])

    ncol = sum(len(_tile_chunks(b, ct)) for b in range(BPC) for ct in range(NCT))
    sum_parts = small.tile([128, ncol], f32, tag="sump")
    max_parts = small.tile([128, ncol], f32, tag="maxp")
    act_scr = scr.tile([128, CHUNK], f32, tag="ascr")
    dve_scr = scr.tile([128, CHUNK], f32, tag="dscr")

    # ---- phase A: stream x, per-channel sum (ScalarE) + max (VectorE).
    # DVE max runs as tensor_scalar+accum (2x fp32 mode) rather than
    # tensor_reduce (1x). For the tapered last chunks the sum moves to DVE
    # too, so ACT and DVE drain the final tile in balance.
    col0 = {}
    col = 0
    for b in range(BPC):
        for ct in range(NCT):
            chunks = _tile_chunks(b, ct)
            col0[(b, ct)] = (col, len(chunks))
            off = 0
            for csz in chunks:
                if csz <= 2048:
                    xt = inp.tile([128, 2048], f32, tag="txt", name="txt", bufs=3)
                else:
                    xt = inp.tile([128, CHUNK], f32, tag="xt", name="xt")
                nc.sync.dma_start(
                    out=xt[:, 0:csz],
                    in_=x[b, ct * 128:(ct + 1) * 128, off:off + csz],
                )
                if csz > 2048:
                    nc.scalar.activation(
                        act_scr[:, 0:csz], xt[:, 0:csz], AF.Copy,
                        accum_out=sum_parts[:, col:col + 1],
                    )
                else:
                    nc.vector.tensor_scalar(
                        out=dve_scr[:, 0:csz], in0=xt[:, 0:csz],
                        scalar1=1.0, scalar2=None, op0=ALU.mult, op1=ALU.add,
                        accum_out=sum_parts[:, col:col + 1],
                    )
                nc.vector.tensor_scalar(
                    out=dve_scr[:, 0:csz], in0=xt[:, 0:csz],
                    scalar1=1.0, scalar2=None, op0=ALU.mult, op1=ALU.max,
                    accum_out=max_parts[:, col:col + 1],
                )
                off += csz
                col += 1

    # ---- combine partials -> (128, BPC) per channel-tile
    sum_c, max_c = [], []
    for ct in range(NCT):
        sum_c.append(small.tile([128, BPC], f32, tag=f"sumc{ct}", name=f"sumc{ct}"))
        max_c.append(small.tile([128, BPC], f32, tag=f"maxc{ct}", name=f"maxc{ct}"))
    for b in range(BPC):
        for ct in range(NCT):
            c0, n = col0[(b, ct)]
            nc.vector.tensor_reduce(
                sum_c[ct][:, b:b + 1], sum_parts[:, c0:c0 + n], axis=AX.X, op=ALU.add)
            nc.vector.tensor_reduce(
                max_c[ct][:, b:b + 1], max_parts[:, c0:c0 + n], axis=AX.X, op=ALU.max)

    # ---- y.T = sum @ (fc1.T/S) + max @ fc2.T + bsum   (batch on partitions)
    yT_ps = psY.tile([BPC, DIM], f32, tag="yT")
    nc.tensor.matmul(yT_ps[:], sum_c[0][:], fc1t_sb[0][:], start=True, stop=False)
    nc.tensor.matmul(yT_ps[:], max_c[0][:], fc2t_sb[0][:], start=False, stop=False)
    nc.tensor.matmul(yT_ps[:], ones14[:], bsum_sb[:], start=False, stop=False)
    nc.tensor.matmul(yT_ps[:], sum_c[1][:], fc1t_sb[1][:], start=False, stop=False)
    nc.tensor.matmul(yT_ps[:], max_c[1][:], fc2t_sb[1][:], start=False, stop=True)

    # ---- groupnorm stats (free-axis reduce over channels)
    yT_sb = small.tile([BPC, DIM], f32, tag="yT_sb")
    y2 = small.tile([BPC, DIM], f32, tag="y2")
    s1 = small.tile([BPC, 1], f32, tag="s1")
    s2 = small.tile([BPC, 1], f32, tag="s2")
    nc.scalar.activation(yT_sb[:], yT_ps[:], AF.Copy, accum_out=s1[:])
    nc.scalar.activation(y2[:], yT_ps[:], AF.Square, accum_out=s2[:])

    mu = small.tile([BPC, 1], f32, tag="mu")
    mu2 = small.tile([BPC, 1], f32, tag="mu2")
    ex2e = small.tile([BPC, 1], f32, tag="ex2e")
    veps = small.tile([BPC, 1], f32, tag="veps")
    nc.vector.tensor_scalar(out=mu[:], in0=s1[:], scalar1=1.0 / DIM, scalar2=None, op0=ALU.mult)
    nc.vector.tensor_tensor(out=mu2[:], in0=mu[:], in1=mu[:], op=ALU.mult)
    nc.vector.tensor_scalar(out=ex2e[:], in0=s2[:], scalar1=1.0 / DIM, scalar2=EPS, op0=ALU.mult, op1=ALU.add)
    nc.vector.tensor_tensor(out=veps[:], in0=ex2e[:], in1=mu2[:], op=ALU.subtract)
    # rstd = 1/sqrt(veps): fast-inverse-sqrt seed + 3 Newton steps (pure DVE,
    # avoids the sqrt activation-table load)
    magic = small.tile([BPC, 1], i32, tag="magic")
    shift1 = small.tile([BPC, 1], i32, tag="shift1")
    ihalf = small.tile([BPC, 1], i32, tag="ihalf")
    nc.vector.memset(magic[:], 0x5F3759DF)
    nc.vector.memset(shift1[:], 1)
    nc.vector.tensor_tensor(out=ihalf[:], in0=veps[:].bitcast(i32), in1=shift1[:],
                            op=ALU.arith_shift_right)
    r = small.tile([BPC, 1], f32, tag="rseed")
    nc.vector.tensor_tensor(out=r[:].bitcast(i32), in0=magic[:], in1=ihalf[:],
                            op=ALU.subtract)
    for it in range(2):
        ra = small.tile([BPC, 1], f32, tag=f"ra{it}")
        rb = small.tile([BPC, 1], f32, tag=f"rb{it}")
        rc = small.tile([BPC, 1], f32, tag=f"rc{it}")
        rn = small.tile([BPC, 1], f32, tag=f"rn{it}")
        nc.vector.tensor_tensor(out=ra[:], in0=r[:], in1=r[:], op=ALU.mult)
        nc.vector.tensor_tensor(out=rb[:], in0=veps[:], in1=ra[:], op=ALU.mult)
        nc.vector.tensor_scalar(out=rc[:], in0=rb[:], scalar1=-0.5, scalar2=1.5, op0=ALU.mult, op1=ALU.add)
        nc.vector.tensor_tensor(out=rn[:], in0=r[:], in1=rc[:], op=ALU.mult)
        r = rn
    rstd = r

    # ---- t = (y - mu) * rstd ; q/k/v = t*w_x + b_x (norm affine folded in)
    t_sb = small.tile([BPC, DIM], f32, tag="t_sb")
    nc.vector.tensor_scalar(
        out=t_sb[:], in0=yT_sb[:], scalar1=mu[:], scalar2=rstd[:],
        op0=ALU.subtract, op1=ALU.mult,
    )
    qkv = []
    for i in range(3):
        w_sl = wqkv_sb[:, (2 * i) * DIM:(2 * i + 1) * DIM]
        b_sl = wqkv_sb[:, (2 * i + 1) * DIM:(2 * i + 2) * DIM]
        tt = tmp.tile([BPC, DIM], f32, tag="qkv_tmp")
        rr = small.tile([BPC, DIM], f32, tag=f"qkv{i}")
        nc.vector.tensor_tensor(out=tt[:], in0=t_sb[:], in1=w_sl, op=ALU.mult)
        nc.vector.tensor_tensor(out=rr[:], in0=tt[:], in1=b_sl, op=ALU.add)
        qkv.append(rr)
    qT, kT, vT = qkv

    # ---- v columns + ones column for the attention contraction
    v1_sb = []
    for ct in range(NCT):
        vps = psV.tile([128, BPC], f32, tag="vps")
        nc.tensor.transpose(vps[:], vT[:, ct * 128:(ct + 1) * 128], ident[0:BPC, 0:BPC])
        v1 = small.tile([128, BPC + 1], f32, tag=f"v1_{ct}")
        nc.vector.tensor_copy(v1[:, 0:BPC], vps[:])
        nc.vector.memset(v1[:, BPC:BPC + 1], 1.0)
        v1_sb.append(v1)

    # matmul operands must sit at a quadrant base partition, so pack the
    # per-batch q/k rows (partitions 0..3) onto partition 0 via SBUF DMA
    qk_sep = small.tile([1, 2 * BPC * DIM], f32, tag="qksep")
    nc.sync.dma_start(
        out=qk_sep[0:1, 0:BPC * DIM].rearrange("p (b i) -> p b i", b=BPC),
        in_=qT[:],
    )
    nc.scalar.dma_start(
        out=qk_sep[0:1, BPC * DIM:2 * BPC * DIM].rearrange("p (b i) -> p b i", b=BPC),
        in_=kT[:],
    )

    # ---- attention: Et[e,d] = exp(s*q_d*k_e); out = (Et.T@v)/(Et.T@1)
    ratio_sb = small.tile([128, 2 * BPC], f32, tag="ratio")
    for b in range(BPC):
        q_row = qk_sep[0:1, b * DIM:(b + 1) * DIM]
        # both channel-tiles of Et go in one PSUM bank so a single exp
        # activation covers them
        pp = psP.tile([128, NCT * DIM], f32, tag="pp")
        for ect in range(NCT):
            k_sl = qk_sep[0:1, BPC * DIM + b * DIM + ect * 128:
                           BPC * DIM + b * DIM + (ect + 1) * 128]
            nc.tensor.matmul(pp[:, ect * DIM:(ect + 1) * DIM],
                             k_sl, q_row, start=True, stop=True)
        et = etp.tile([128, NCT * DIM], f32, tag="et")
        nc.scalar.activation(et[:], pp[:], AF.Exp, scale=SCALER)
        for dt in range(2):
            nd = psN.tile([128, 2], f32, tag="nd")
            for ect in range(NCT):
                nc.tensor.matmul(
                    nd[:], et[:, ect * DIM + dt * 128:ect * DIM + (dt + 1) * 128],
                    v1_sb[ect][:, b::(BPC - b)],
                    start=(ect == 0), stop=(ect == NCT - 1),
                )
            col = b * 2 + dt
            nc.vector.reciprocal(ratio_sb[:, col:col + 1], nd[:, 1:2])
            nc.vector.tensor_tensor(
                out=ratio_sb[:, col:col + 1], in0=nd[:, 0:1],
                in1=ratio_sb[:, col:col + 1], op=ALU.mult,
            )

    # ---- transpose to (2*BPC, 128), sigmoid via tanh, store
    ratT = psY.tile([2 * BPC, 128], f32, tag="ratT")
    nc.tensor.transpose(ratT[:], ratio_sb[:], ident[:])
    gate_t = small.tile([2 * BPC, 128], f32, tag="gate_t")
    gate = small.tile([2 * BPC, 128], f32, tag="gate")
    # sigmoid(x) = 0.5*tanh(x/2) + 0.5  (tanh shares the exp table set)
    nc.scalar.activation(gate_t[:], ratT[:], AF.Tanh, scale=0.5)
    nc.vector.tensor_scalar(out=gate[:], in0=gate_t[:], scalar1=0.5, scalar2=0.5,
                            op0=ALU.mult, op1=ALU.add)
    nc.sync.dma_start(out=out[:], in_=gate[:])


def _build():
    nc = bacc.Bacc("TRN2", target_bir_lowering=False, debug=False, num_devices=NCORES)
    x_d = nc.dram_tensor("x", [BPC, DIM, S], f32, kind="ExternalInput")
    fc1t_d = nc.dram_tensor("fc1t", [DIM, DIM], f32, kind="ExternalInput")
    fc2t_d = nc.dram_tensor("fc2t", [DIM, DIM], f32, kind="ExternalInput")
    bsum_d = nc.dram_tensor("bsum", [1, DIM], f32, kind="ExternalInput")
    wqkv_d = nc.dram_tensor("wqkv", [BPC, 6 * DIM], f32, kind="ExternalInput")
    out_d = nc.dram_tensor("out", [2 * BPC, 128], f32, kind="ExternalOutput")
    with tile.TileContext(nc) as tc:
        with ExitStack() as ctx:
            _body(ctx, tc, x_d.ap(), fc1t_d.ap(), fc2t_d.ap(), bsum_d.ap(),
                  wqkv_d.ap(), out_d.ap())
    nc.compile()
    return nc


_NC_CACHE = {}


def get_nc():
    if "nc" not in _NC_CACHE:
        _NC_CACHE["nc"] = _build()
    return _NC_CACHE["nc"]


def make_in_maps(x, fc1_w, fc1_b, fc2_w, fc2_b, norm_w, norm_b, q_w, k_w, v_w):
    x = np.ascontiguousarray(np.asarray(x, dtype=np.float32))
    fc1_w = np.asarray(fc1_w, dtype=np.float32)
    fc2_w = np.asarray(fc2_w, dtype=np.float32)
    # 1/S = 2^-14 is exact in fp32, so folding the mean into fc1_wT is exact
    fc1t = np.ascontiguousarray(fc1_w.T * np.float32(1.0 / S))
    fc2t = np.ascontiguousarray(fc2_w.T)
    bsum = (np.asarray(fc1_b, np.float32) + np.asarray(fc2_b, np.float32)).reshape(1, DIM)
    nw = np.asarray(norm_w, np.float32)
    nb = np.asarray(norm_b, np.float32)
    rows = []
    for sw in (q_w, k_w, v_w):
        sw = np.asarray(sw, np.float32)
        rows.append(nw * sw)
        rows.append(nb * sw)
    wqkv = np.tile(np.concatenate(rows).reshape(1, 6 * DIM), (BPC, 1)).astype(np.float32)
    return [
        {
            "x": x[i * BPC:(i + 1) * BPC].reshape(BPC, DIM, S),
            "fc1t": fc1t,
            "fc2t": fc2t,
            "bsum": bsum,
            "wqkv": wqkv,
        }
        for i in range(NCORES)
    ]


def run(in_maps, trace=False, **kwargs):
    nc = get_nc()
    return run_bass_kernel_spmd(nc, in_maps, list(range(NCORES)), trace=trace, **kwargs)


def kernel(**inputs):
    in_maps = make_in_maps(**inputs)
    res = run(in_maps, trace=False)
    outs = [res.results[i]["out"].reshape(BPC, DIM, 1, 1) for i in range(NCORES)]
    return np.concatenate(outs, axis=0).astype(np.float32)
